# revision 26
# baseline (speedup 1.0000x reference)
"""Trainium2 Bass kernel for nn_AttentionProbe_80891414053184.

Math (reference):
    y  = relu(x @ W1.T + b1)            # (B,S,H) -> (B,S,128)
    y2 = relu(y @ W2.T + b2)            # (B,S,128)
    l  = y2 @ Wq.T + pos*pos_w  (+mask) # (B,S,8) logits
    p  = softmax(l, axis=S)
    v  = y2 @ Wv.T + bv
    out[b] = sum_{s,h} p*v + bias       # (B,1)

Strategy: sequence-parallel over 8 cores (512 positions x 4 batches = 2048
tokens per core).  Each core streams its x-shard (pre-transposed on host to
(H, tokens) so the contraction dim lands on SBUF partitions), runs the MLP +
head projections on-chip, and emits per-(batch, head) partial softmax stats
(-max, Z=sum exp, W=sum exp*v).  The host merges the 8 partial stats with the
standard online-softmax combine and produces the (4,1) output.
"""

import os

import numpy as np

# Problem dims (hardcoded per harness contract).
B, S, H = 4, 4096, 4096
MLP, NH = 128, 8
NCORES = 8
S_SHARD = S // NCORES        # 512 seq positions per core
TOK = B * S_SHARD            # 2048 tokens per core
NT = TOK // 512              # 4 token tiles of 512 (= one batch each)
KCH = H // 128               # 32 contraction chunks

# float32r runs the big matmul at 4x the fp32 rate with slightly relaxed
# precision; toggled off via env for A/B testing.
USE_F32R = os.environ.get("KERNEL_F32R", "1") == "1"

_cache = {}


def _build_nc(h, use_f32r):
    import concourse.mybir as mybir
    import concourse.tile as tile
    from concourse import bacc
    from concourse.tile import add_dep_helper

    f32 = mybir.dt.float32
    fmm = mybir.dt.float32r if use_f32r else f32
    kch = h // 128

    # Bacc (not bare Bass): its finalize() runs move_matmul_waits_to_ldweights
    # and generate_event_semaphores, which split multi-sem waits to satisfy
    # TRN2's one-wait-per-instruction encoding limit.
    nc = bacc.Bacc()
    xt_d = nc.dram_tensor("xt", [h, TOK], fmm, kind="ExternalInput")
    w1_d = nc.dram_tensor("w1s", [128, kch, MLP], fmm, kind="ExternalInput")
    # cw: [w2t | wqt | wvt | b1 | b2] on 128 partitions; ca: [addt | bv] on 8.
    cw_d = nc.dram_tensor("cw", [MLP, MLP + 2 * NH + 2], f32,
                          kind="ExternalInput")
    ca_d = nc.dram_tensor("ca", [NH, TOK + 1], f32, kind="ExternalInput")
    st_d = nc.dram_tensor("stats", [NH, NT, 3], f32, kind="ExternalOutput")

    AF = mybir.ActivationFunctionType
    AX = mybir.AxisListType
    OP = mybir.AluOpType
    CB1 = MLP + 2 * NH          # col index of b1 in cw
    CQ = MLP                    # wqt cols
    CV = MLP + NH               # wvt cols

    with tile.TileContext(nc) as tc:
        with (
            tc.tile_pool(name="const", bufs=1) as const,
            tc.tile_pool(name="xp", bufs=4) as xp,
            tc.tile_pool(name="yp", bufs=4) as yp,
            tc.tile_pool(name="y2p", bufs=4) as y2p,
            tc.tile_pool(name="smallp", bufs=4) as smallp,
            tc.tile_pool(name="statsp", bufs=1) as statsp,
            tc.tile_pool(name="ps_y", bufs=4, space="PSUM") as ps_y,
            tc.tile_pool(name="ps_y2", bufs=1, space="PSUM") as ps_y2,
            tc.tile_pool(name="ps_q", bufs=1, space="PSUM") as ps_q,
            tc.tile_pool(name="ps_v", bufs=1, space="PSUM") as ps_v,
            tc.tile_pool(name="ps_warm", bufs=1, space="PSUM") as ps_warm,
        ):
            w1_sb = const.tile([128, kch, MLP], fmm)
            nc.sync.dma_start(out=w1_sb[:], in_=w1_d[:])
            cw_sb = const.tile([MLP, MLP + 2 * NH + 2], f32)
            nc.sync.dma_start(out=cw_sb[:], in_=cw_d[:])
            ca_sb = const.tile([NH, TOK + 1], f32)
            nc.sync.dma_start(out=ca_sb[:], in_=ca_d[:])

            stats_sb = statsp.tile([NH, NT, 3], f32)

            # --- Warmup / staging -------------------------------------------
            # HW instruction encodings hold a single sem-wait slot (LDWEIGHTS,
            # DMA).  Each engine "observes" every const-DMA lane once, via
            # throwaway ops emitted before the real consumers, so steady-state
            # instructions need at most one new wait.
            warm_ps = ps_warm.tile([128, NH], f32, name="warm_ps")
            nc.tensor.matmul(warm_ps[0:NH, 0:NH], cw_sb[:, 0:NH],
                             cw_sb[:, 0:NH], start=True, stop=True)
            warm_pe_last = nc.tensor.matmul(warm_ps[:, 0:NH], w1_sb[:, 0, :],
                                            w1_sb[:, 0, 0:NH],
                                            start=True, stop=True)
            warm_act = const.tile([MLP, 1], f32)
            nc.scalar.copy(out=warm_act[:], in_=cw_sb[:, CB1:CB1 + 1])
            warm_act8 = const.tile([NH, 1], f32)
            nc.scalar.copy(out=warm_act8[:], in_=ca_sb[:, TOK:TOK + 1])
            warm_dve = const.tile([NH, 1], f32)
            nc.vector.tensor_copy(out=warm_dve[:], in_=ca_sb[:, 0:1])

            # Layer 1: yT[t] (128, 512) += W1T_chunk.T @ xT_chunk, k-accumulated
            psum_y = []
            for t in range(NT):
                y_ps = ps_y.tile([128, 512], f32, tag="y", name=f"y_ps{t}")
                psum_y.append(y_ps)

            # DMA instructions hold a single sem-wait slot.  A recycled x-slot
            # DMA needs both a PE wait (WAR on the slot\'s last matmul reader)
            # and a DMA-lane wait; park the PE wait on a tiny Pool-engine
            # escort op ordered just before the DMA.
            mm_last = {}
            for k in range(kch):
                x_sb = xp.tile([128, TOK], fmm, tag="x", name=f"x_sb{k}")
                dma = nc.gpsimd.dma_start(out=x_sb[:],
                                          in_=xt_d[k * 128:(k + 1) * 128, :])
                if k >= 4:
                    esc_t = const.tile([1, 1], f32, name=f"esc_x{k}")
                    esc = nc.gpsimd.memset(esc_t[:], 0.0)
                    add_dep_helper(esc.ins, mm_last[k - 4].ins, sync=True,
                                   reason="escort PE wait for x slot WAR")
                    add_dep_helper(dma.ins, esc.ins, sync=False,
                                   reason="escort precedes dma")
                for t in range(NT):
                    mm = nc.tensor.matmul(
                        psum_y[t][:],
                        w1_sb[:, k, :],
                        x_sb[:, t * 512:(t + 1) * 512],
                        start=(k == 0),
                        stop=(k == kch - 1),
                    )
                    if k == 0 and t == 0:
                        # Force warmups ahead of the first real matmul so its
                        # weight-lane wait is already observed.
                        add_dep_helper(mm.ins, warm_pe_last.ins, sync=False,
                                       reason="warmup before first matmul")
                mm_last[k] = mm

            for t in range(NT):
                ts_ = slice(t * 512, (t + 1) * 512)
                y_sb = yp.tile([128, 512], f32, tag="ysb", name=f"y_sb{t}")
                nc.scalar.activation(out=y_sb[:], in_=psum_y[t][:], func=AF.Relu,
                                     bias=cw_sb[:, CB1:CB1 + 1], scale=1.0)
                y2_ps = ps_y2.tile([128, 512], f32, tag="y2", name=f"y2_ps{t}")
                nc.tensor.matmul(y2_ps[:], cw_sb[:, 0:MLP], y_sb[:],
                                 start=True, stop=True)
                y2_sb = y2p.tile([128, 512], f32, tag="y2sb", name=f"y2_sb{t}")
                nc.scalar.activation(out=y2_sb[:], in_=y2_ps[:], func=AF.Relu,
                                     bias=cw_sb[:, CB1 + 1:CB1 + 2], scale=1.0)
                q_ps = ps_q.tile([NH, 512], f32, tag="q", name=f"q_ps{t}")
                nc.tensor.matmul(q_ps[:], cw_sb[:, CQ:CQ + NH], y2_sb[:],
                                 start=True, stop=True)
                v_ps = ps_v.tile([NH, 512], f32, tag="v", name=f"v_ps{t}")
                nc.tensor.matmul(v_ps[:], cw_sb[:, CV:CV + NH], y2_sb[:],
                                 start=True, stop=True)

                # PSUM -> SBUF moves ride ACT so the q/v psum slot-release dep
                # merges with the y2_sb dep into a single ACT sem wait on PE.
                l0_sb = smallp.tile([NH, 512], f32, tag="l0", name=f"l0_sb{t}")
                nc.scalar.copy(out=l0_sb[:], in_=q_ps[:])
                v_sb = smallp.tile([NH, 512], f32, tag="vs", name=f"v_sb{t}")
                # v = v_ps + bv, fused into the ACT move
                nc.scalar.activation(out=v_sb[:], in_=v_ps[:], func=AF.Identity,
                                     bias=ca_sb[:, TOK:TOK + 1], scale=1.0)
                l_sb = smallp.tile([NH, 512], f32, tag="l", name=f"l_sb{t}")
                nc.vector.tensor_add(out=l_sb[:], in0=l0_sb[:],
                                     in1=ca_sb[:, ts_])
                # stats[:, t, 0] = -max_s l
                nc.vector.tensor_reduce(out=stats_sb[:, t, 0:1], in_=l_sb[:],
                                        axis=AX.X, op=OP.max, negate=True)
                e_sb = smallp.tile([NH, 512], f32, tag="e", name=f"e_sb{t}")
                # e = exp(l - max); stats[:, t, 1] = Z = sum e
                exp_op = nc.scalar.activation(out=e_sb[:], in_=l_sb[:],
                                              func=AF.Exp,
                                              bias=stats_sb[:, t, 0:1],
                                              scale=1.0,
                                              accum_out=stats_sb[:, t, 1:2])
                ev_sb = smallp.tile([NH, 512], f32, tag="ev", name=f"ev_sb{t}")
                nc.vector.tensor_mul(out=ev_sb[:], in0=e_sb[:], in1=v_sb[:])
                # stats[:, t, 2] = W = sum e*v
                ttr = nc.vector.tensor_reduce(out=stats_sb[:, t, 2:3],
                                              in_=ev_sb[:], axis=AX.X,
                                              op=OP.add)
                if t == NT - 1:
                    exp_last, ttr_last = exp_op, ttr

            # Escort the ACT/DVE waits of the stats store (1 wait/DMA limit).
            esc_ta = const.tile([1, 1], f32, name="esc_sta")
            esc_a = nc.gpsimd.memset(esc_ta[:], 0.0)
            add_dep_helper(esc_a.ins, ttr_last.ins, sync=True,
                           reason="escort DVE wait for stats store")
            esc_tb = const.tile([1, 1], f32, name="esc_stb")
            esc_b = nc.gpsimd.memset(esc_tb[:], 0.0)
            add_dep_helper(esc_b.ins, exp_last.ins, sync=True,
                           reason="escort ACT wait for stats store")
            st_dma = nc.gpsimd.dma_start(out=st_d[:], in_=stats_sb[:])
            add_dep_helper(st_dma.ins, esc_a.ins, sync=False,
                           reason="escort precedes stats store")
            add_dep_helper(st_dma.ins, esc_b.ins, sync=False,
                           reason="escort precedes stats store")

    nc.finalize()
    return nc


def get_nc(h=H, use_f32r=USE_F32R):
    key = (h, use_f32r)
    if key not in _cache:
        _cache[key] = _build_nc(h, use_f32r)
    return _cache[key]


def make_core_inputs(x, mask, W1, b1, W2, b2, Wq, Wv, bv, pos_w, bias):
    """Host-side shard + transpose. Returns list of 8 in_maps."""
    h = x.shape[2]
    kch = h // 128
    w1s = np.ascontiguousarray(
        W1.reshape(MLP, kch, 128).transpose(2, 1, 0)).astype(np.float32)
    cw = np.zeros((MLP, MLP + 2 * NH + 2), dtype=np.float32)
    cw[:, 0:MLP] = W2.T
    cw[:, MLP:MLP + NH] = Wq.T
    cw[:, MLP + NH:MLP + 2 * NH] = Wv.T
    cw[:, MLP + 2 * NH] = b1
    cw[:, MLP + 2 * NH + 1] = b2
    pos = np.arange(S, dtype=np.float32)
    maskadd = np.where(mask == 0, np.float32(-1e9), np.float32(0.0))  # (B,S)

    in_maps = []
    for c in range(NCORES):
        sl = slice(c * S_SHARD, (c + 1) * S_SHARD)
        xt = np.ascontiguousarray(
            x[:, sl, :].transpose(2, 0, 1).reshape(h, TOK)).astype(np.float32)
        ca = np.empty((NH, TOK + 1), dtype=np.float32)
        ca[:, 0:TOK] = (pos_w.astype(np.float32)[:, None, None]
                        * pos[sl][None, None, :]
                        + maskadd[None, :, sl]).reshape(NH, TOK)
        ca[:, TOK] = bv
        in_maps.append({"xt": xt, "w1s": w1s, "cw": cw, "ca": ca})
    return in_maps


def merge_stats(stats_all, bias):
    """stats_all: (NCORES, NH, B, 3) with [-m, Z, W] -> (B, 1) output."""
    st = np.asarray(stats_all, dtype=np.float64)
    m = -st[..., 0]          # (C, NH, B)
    Z = st[..., 1]
    W = st[..., 2]
    M = m.max(axis=0)        # (NH, B)
    alpha = np.exp(m - M[None])
    Zg = (alpha * Z).sum(axis=0)
    Wg = (alpha * W).sum(axis=0)
    out = (Wg / Zg).sum(axis=0)          # (B,)
    return (out[:, None] + np.float64(bias.reshape(1)[0])).astype(np.float32)


def kernel(x, mask, W1, b1, W2, b2, Wq, Wv, bv, pos_w, bias, _trace=False):
    from concourse.bass_utils import run_bass_kernel_spmd

    x = np.asarray(x, dtype=np.float32)
    in_maps = make_core_inputs(x, np.asarray(mask), *(np.asarray(a) for a in
                               (W1, b1, W2, b2, Wq, Wv, bv, pos_w, bias)))
    nc = get_nc()
    res = run_bass_kernel_spmd(nc, in_maps, core_ids=list(range(NCORES)),
                               trace=_trace)
    stats_all = np.stack([r["stats"] for r in res.results])  # (C, NH, NT, 3)
    out = merge_stats(stats_all, np.asarray(bias))
    if _trace:
        kernel.last_result = res
    return out


# revision 27
# speedup vs baseline: 1.5446x; 1.5446x over previous
"""Trainium2 Bass kernel for nn_AttentionProbe_80891414053184.

Math (reference):
    y  = relu(x @ W1.T + b1)            # (B,S,H) -> (B,S,128)
    y2 = relu(y @ W2.T + b2)            # (B,S,128)
    l  = y2 @ Wq.T + pos*pos_w  (+mask) # (B,S,8) logits
    p  = softmax(l, axis=S)
    v  = y2 @ Wv.T + bv
    out[b] = sum_{s,h} p*v + bias       # (B,1)

Strategy: sequence-parallel over 8 cores (512 positions x 4 batches = 2048
tokens per core).  Each core streams its x-shard (pre-transposed on host to
(H, tokens) so the contraction dim lands on SBUF partitions), runs the MLP +
head projections on-chip, and emits per-(batch, head) partial softmax stats
(-max, Z=sum exp, W=sum exp*v).  The host merges the 8 partial stats with the
standard online-softmax combine and produces the (4,1) output.
"""

import os

import numpy as np

# Problem dims (hardcoded per harness contract).
B, S, H = 4, 4096, 4096
MLP, NH = 128, 8
NCORES = 8
S_SHARD = S // NCORES        # 512 seq positions per core
TOK = B * S_SHARD            # 2048 tokens per core
NT = TOK // 512              # 4 token tiles of 512 (= one batch each)
KCH = H // 128               # 32 contraction chunks

# Layer-1 operand dtype: bf16 halves HBM traffic for x AND runs the PE at
# 1 cycle/row (fp32 takes 4, f32r ~2-3).  Measured end-to-end output error of
# the bf16 path vs the fp32 reference is ~2e-4 (softmax averaging washes out
# the rounding noise).  Override with KERNEL_DT=f32|f32r|bf16 for A/B tests.
KERNEL_DT = os.environ.get("KERNEL_DT",
                           "f32r" if os.environ.get("KERNEL_F32R") == "1"
                           else "bf16")

_cache = {}


def _build_nc(h, dt_name):
    import concourse.mybir as mybir
    import concourse.tile as tile
    from concourse import bacc
    from concourse.tile import add_dep_helper

    f32 = mybir.dt.float32
    fmm = {"f32": f32, "f32r": mybir.dt.float32r,
           "bf16": mybir.dt.bfloat16}[dt_name]
    kch = h // 128

    # Bacc (not bare Bass): its finalize() runs move_matmul_waits_to_ldweights
    # and generate_event_semaphores, which split multi-sem waits to satisfy
    # TRN2's one-wait-per-instruction encoding limit.
    nc = bacc.Bacc()
    xt_d = nc.dram_tensor("xt", [h, TOK], fmm, kind="ExternalInput")
    w1_d = nc.dram_tensor("w1s", [128, kch, MLP], fmm, kind="ExternalInput")
    # cw: [w2t | wqt | wvt | b1 | b2] on 128 partitions; ca: [addt | bv] on 8.
    cw_d = nc.dram_tensor("cw", [MLP, MLP + 2 * NH + 2], f32,
                          kind="ExternalInput")
    ca_d = nc.dram_tensor("ca", [NH, TOK + 1], f32, kind="ExternalInput")
    st_d = nc.dram_tensor("stats", [NH, NT, 3], f32, kind="ExternalOutput")

    AF = mybir.ActivationFunctionType
    AX = mybir.AxisListType
    OP = mybir.AluOpType
    CB1 = MLP + 2 * NH          # col index of b1 in cw
    CQ = MLP                    # wqt cols
    CV = MLP + NH               # wvt cols

    with tile.TileContext(nc) as tc:
        with (
            tc.tile_pool(name="const", bufs=1) as const,
            tc.tile_pool(name="xp", bufs=4) as xp,
            tc.tile_pool(name="yp", bufs=4) as yp,
            tc.tile_pool(name="y2p", bufs=4) as y2p,
            tc.tile_pool(name="smallp", bufs=4) as smallp,
            tc.tile_pool(name="statsp", bufs=1) as statsp,
            tc.tile_pool(name="ps_y", bufs=4, space="PSUM") as ps_y,
            tc.tile_pool(name="ps_y2", bufs=1, space="PSUM") as ps_y2,
            tc.tile_pool(name="ps_q", bufs=1, space="PSUM") as ps_q,
            tc.tile_pool(name="ps_v", bufs=1, space="PSUM") as ps_v,
            tc.tile_pool(name="ps_warm", bufs=1, space="PSUM") as ps_warm,
        ):
            w1_sb = const.tile([128, kch, MLP], fmm)
            nc.sync.dma_start(out=w1_sb[:], in_=w1_d[:])
            cw_sb = const.tile([MLP, MLP + 2 * NH + 2], f32)
            nc.sync.dma_start(out=cw_sb[:], in_=cw_d[:])
            ca_sb = const.tile([NH, TOK + 1], f32)
            nc.sync.dma_start(out=ca_sb[:], in_=ca_d[:])

            stats_sb = statsp.tile([NH, NT, 3], f32)

            # --- Warmup / staging -------------------------------------------
            # HW instruction encodings hold a single sem-wait slot (LDWEIGHTS,
            # DMA).  Each engine "observes" every const-DMA lane once, via
            # throwaway ops emitted before the real consumers, so steady-state
            # instructions need at most one new wait.
            warm_ps = ps_warm.tile([128, NH], f32, name="warm_ps")
            nc.tensor.matmul(warm_ps[0:NH, 0:NH], cw_sb[:, 0:NH],
                             cw_sb[:, 0:NH], start=True, stop=True)
            warm_pe_last = nc.tensor.matmul(warm_ps[:, 0:NH], w1_sb[:, 0, :],
                                            w1_sb[:, 0, 0:NH],
                                            start=True, stop=True)
            warm_act = const.tile([MLP, 1], f32)
            nc.scalar.copy(out=warm_act[:], in_=cw_sb[:, CB1:CB1 + 1])
            warm_act8 = const.tile([NH, 1], f32)
            nc.scalar.copy(out=warm_act8[:], in_=ca_sb[:, TOK:TOK + 1])
            warm_dve = const.tile([NH, 1], f32)
            nc.vector.tensor_copy(out=warm_dve[:], in_=ca_sb[:, 0:1])

            # Layer 1: yT[t] (128, 512) += W1T_chunk.T @ xT_chunk, k-accumulated
            psum_y = []
            for t in range(NT):
                y_ps = ps_y.tile([128, 512], f32, tag="y", name=f"y_ps{t}")
                psum_y.append(y_ps)

            # DMA instructions hold a single sem-wait slot.  A recycled x-slot
            # DMA needs both a PE wait (WAR on the slot\'s last matmul reader)
            # and a DMA-lane wait; park the PE wait on a tiny Pool-engine
            # escort op ordered just before the DMA.
            mm_last = {}
            for k in range(kch):
                x_sb = xp.tile([128, TOK], fmm, tag="x", name=f"x_sb{k}")
                dma = nc.gpsimd.dma_start(out=x_sb[:],
                                          in_=xt_d[k * 128:(k + 1) * 128, :])
                if k >= 4:
                    esc_t = const.tile([1, 1], f32, name=f"esc_x{k}")
                    esc = nc.gpsimd.memset(esc_t[:], 0.0)
                    add_dep_helper(esc.ins, mm_last[k - 4].ins, sync=True,
                                   reason="escort PE wait for x slot WAR")
                    add_dep_helper(dma.ins, esc.ins, sync=False,
                                   reason="escort precedes dma")
                for t in range(NT):
                    mm = nc.tensor.matmul(
                        psum_y[t][:],
                        w1_sb[:, k, :],
                        x_sb[:, t * 512:(t + 1) * 512],
                        start=(k == 0),
                        stop=(k == kch - 1),
                    )
                    if k == 0 and t == 0:
                        # Force warmups ahead of the first real matmul so its
                        # weight-lane wait is already observed.
                        add_dep_helper(mm.ins, warm_pe_last.ins, sync=False,
                                       reason="warmup before first matmul")
                mm_last[k] = mm

            for t in range(NT):
                ts_ = slice(t * 512, (t + 1) * 512)
                y_sb = yp.tile([128, 512], f32, tag="ysb", name=f"y_sb{t}")
                nc.scalar.activation(out=y_sb[:], in_=psum_y[t][:], func=AF.Relu,
                                     bias=cw_sb[:, CB1:CB1 + 1], scale=1.0)
                y2_ps = ps_y2.tile([128, 512], f32, tag="y2", name=f"y2_ps{t}")
                nc.tensor.matmul(y2_ps[:], cw_sb[:, 0:MLP], y_sb[:],
                                 start=True, stop=True)
                y2_sb = y2p.tile([128, 512], f32, tag="y2sb", name=f"y2_sb{t}")
                nc.scalar.activation(out=y2_sb[:], in_=y2_ps[:], func=AF.Relu,
                                     bias=cw_sb[:, CB1 + 1:CB1 + 2], scale=1.0)
                q_ps = ps_q.tile([NH, 512], f32, tag="q", name=f"q_ps{t}")
                nc.tensor.matmul(q_ps[:], cw_sb[:, CQ:CQ + NH], y2_sb[:],
                                 start=True, stop=True)
                v_ps = ps_v.tile([NH, 512], f32, tag="v", name=f"v_ps{t}")
                nc.tensor.matmul(v_ps[:], cw_sb[:, CV:CV + NH], y2_sb[:],
                                 start=True, stop=True)

                # PSUM -> SBUF moves ride ACT so the q/v psum slot-release dep
                # merges with the y2_sb dep into a single ACT sem wait on PE.
                l0_sb = smallp.tile([NH, 512], f32, tag="l0", name=f"l0_sb{t}")
                nc.scalar.copy(out=l0_sb[:], in_=q_ps[:])
                v_sb = smallp.tile([NH, 512], f32, tag="vs", name=f"v_sb{t}")
                # v = v_ps + bv, fused into the ACT move
                nc.scalar.activation(out=v_sb[:], in_=v_ps[:], func=AF.Identity,
                                     bias=ca_sb[:, TOK:TOK + 1], scale=1.0)
                l_sb = smallp.tile([NH, 512], f32, tag="l", name=f"l_sb{t}")
                nc.vector.tensor_add(out=l_sb[:], in0=l0_sb[:],
                                     in1=ca_sb[:, ts_])
                # stats[:, t, 0] = -max_s l
                nc.vector.tensor_reduce(out=stats_sb[:, t, 0:1], in_=l_sb[:],
                                        axis=AX.X, op=OP.max, negate=True)
                e_sb = smallp.tile([NH, 512], f32, tag="e", name=f"e_sb{t}")
                # e = exp(l - max); stats[:, t, 1] = Z = sum e
                exp_op = nc.scalar.activation(out=e_sb[:], in_=l_sb[:],
                                              func=AF.Exp,
                                              bias=stats_sb[:, t, 0:1],
                                              scale=1.0,
                                              accum_out=stats_sb[:, t, 1:2])
                ev_sb = smallp.tile([NH, 512], f32, tag="ev", name=f"ev_sb{t}")
                nc.vector.tensor_mul(out=ev_sb[:], in0=e_sb[:], in1=v_sb[:])
                # stats[:, t, 2] = W = sum e*v
                ttr = nc.vector.tensor_reduce(out=stats_sb[:, t, 2:3],
                                              in_=ev_sb[:], axis=AX.X,
                                              op=OP.add)
                if t == NT - 1:
                    exp_last, ttr_last = exp_op, ttr

            # Escort the ACT/DVE waits of the stats store (1 wait/DMA limit).
            esc_ta = const.tile([1, 1], f32, name="esc_sta")
            esc_a = nc.gpsimd.memset(esc_ta[:], 0.0)
            add_dep_helper(esc_a.ins, ttr_last.ins, sync=True,
                           reason="escort DVE wait for stats store")
            esc_tb = const.tile([1, 1], f32, name="esc_stb")
            esc_b = nc.gpsimd.memset(esc_tb[:], 0.0)
            add_dep_helper(esc_b.ins, exp_last.ins, sync=True,
                           reason="escort ACT wait for stats store")
            st_dma = nc.gpsimd.dma_start(out=st_d[:], in_=stats_sb[:])
            add_dep_helper(st_dma.ins, esc_a.ins, sync=False,
                           reason="escort precedes stats store")
            add_dep_helper(st_dma.ins, esc_b.ins, sync=False,
                           reason="escort precedes stats store")

    nc.finalize()
    return nc


def get_nc(h=H, dt_name=None, use_f32r=None):
    if dt_name is None:
        dt_name = ("f32r" if use_f32r else "f32") if use_f32r is not None \
            else KERNEL_DT
    key = (h, dt_name)
    if key not in _cache:
        _cache[key] = _build_nc(h, dt_name)
    return _cache[key]


def make_core_inputs(x, mask, W1, b1, W2, b2, Wq, Wv, bv, pos_w, bias):
    """Host-side shard + transpose. Returns list of 8 in_maps."""
    h = x.shape[2]
    kch = h // 128
    w1s = np.ascontiguousarray(
        W1.reshape(MLP, kch, 128).transpose(2, 1, 0)).astype(np.float32)
    cw = np.zeros((MLP, MLP + 2 * NH + 2), dtype=np.float32)
    cw[:, 0:MLP] = W2.T
    cw[:, MLP:MLP + NH] = Wq.T
    cw[:, MLP + NH:MLP + 2 * NH] = Wv.T
    cw[:, MLP + 2 * NH] = b1
    cw[:, MLP + 2 * NH + 1] = b2
    pos = np.arange(S, dtype=np.float32)
    maskadd = np.where(mask == 0, np.float32(-1e9), np.float32(0.0))  # (B,S)

    if KERNEL_DT == "bf16":
        import ml_dtypes
        mmdt = ml_dtypes.bfloat16
    else:
        mmdt = np.float32
    w1s = w1s.astype(mmdt)

    in_maps = []
    for c in range(NCORES):
        sl = slice(c * S_SHARD, (c + 1) * S_SHARD)
        xt = np.ascontiguousarray(
            x[:, sl, :].astype(mmdt).transpose(2, 0, 1).reshape(h, TOK))
        ca = np.empty((NH, TOK + 1), dtype=np.float32)
        ca[:, 0:TOK] = (pos_w.astype(np.float32)[:, None, None]
                        * pos[sl][None, None, :]
                        + maskadd[None, :, sl]).reshape(NH, TOK)
        ca[:, TOK] = bv
        in_maps.append({"xt": xt, "w1s": w1s, "cw": cw, "ca": ca})
    return in_maps


def merge_stats(stats_all, bias):
    """stats_all: (NCORES, NH, B, 3) with [-m, Z, W] -> (B, 1) output."""
    st = np.asarray(stats_all, dtype=np.float64)
    m = -st[..., 0]          # (C, NH, B)
    Z = st[..., 1]
    W = st[..., 2]
    M = m.max(axis=0)        # (NH, B)
    alpha = np.exp(m - M[None])
    Zg = (alpha * Z).sum(axis=0)
    Wg = (alpha * W).sum(axis=0)
    out = (Wg / Zg).sum(axis=0)          # (B,)
    return (out[:, None] + np.float64(bias.reshape(1)[0])).astype(np.float32)


def kernel(x, mask, W1, b1, W2, b2, Wq, Wv, bv, pos_w, bias, _trace=False):
    from concourse.bass_utils import run_bass_kernel_spmd

    x = np.asarray(x, dtype=np.float32)
    in_maps = make_core_inputs(x, np.asarray(mask), *(np.asarray(a) for a in
                               (W1, b1, W2, b2, Wq, Wv, bv, pos_w, bias)))
    nc = get_nc()
    res = run_bass_kernel_spmd(nc, in_maps, core_ids=list(range(NCORES)),
                               trace=_trace)
    stats_all = np.stack([r["stats"] for r in res.results])  # (C, NH, NT, 3)
    out = merge_stats(stats_all, np.asarray(bias))
    if _trace:
        kernel.last_result = res
    return out
